# revision 11
# baseline (speedup 1.0000x reference)
"""Trainium2 Bass kernel for single-head attention:
out = softmax((x@Wq)(x@Wk)^T / sqrt(D)) (x@Wv) @ Wout + bout, per batch.

Shapes: x [4, 2048, 1024], Wqkv [1024, 3072], Wout [1024, 1024].

Sharding: 8 cores = 4 batches x 2 query-halves. Each core gets the full
sequence of its batch (rotated so its 1024 queries are rows 0:1023 --
attention is permutation-invariant over key/value positions), computes
QKV for the whole sequence, attention + output projection for its 1024
queries, and writes a [1024, 1024] slice of the output. No collectives.

x and the weight matrices are pre-cast to fp16 host-side (same rounding
the on-chip cast would apply); all matmuls run fp16 (full PE rate) with
fp32 PSUM accumulation. x^T is produced by DMA xbar transpose. Softmax
exp uses a constant -4 logit shift so fp16 never overflows (the shift
cancels in the normalization).
"""

import sys

if "/opt/trn_rl_repo" not in sys.path:
    sys.path.insert(0, "/opt/trn_rl_repo")

import numpy as np

import concourse.bass as bass
import concourse.mybir as mybir
from concourse import bacc
from concourse.tile import TileContext

P = 128
D = 1024          # d_model
S = 2048          # full sequence per batch
SQ = 1024         # queries per core
DC = D // P       # 8 d-chunks
ST = S // P       # 16 sequence tiles
NQ = SQ // 512    # 2 query 512-chunks
NK = S // 512     # 4 sequence 512-chunks
SQT = SQ // P     # 8 query 128-tiles

F32 = mybir.dt.float32
F16 = mybir.dt.float16
EXP_SHIFT = -4.0  # softmax logit shift (cancels in normalization)


def _build_core_program():
    nc = bacc.Bacc()

    x_d = nc.dram_tensor("x16", [S, D], F16, kind="ExternalInput")
    wqkv_d = nc.dram_tensor("Wqkv16", [D, 3 * D], F16, kind="ExternalInput")
    bqkv_d = nc.dram_tensor("bqkv", [3 * D], F32, kind="ExternalInput")
    wout_d = nc.dram_tensor("Wout16", [D, D], F16, kind="ExternalInput")
    bout_d = nc.dram_tensor("bout", [D], F32, kind="ExternalInput")
    out_d = nc.dram_tensor("out", [SQ, D], F32, kind="ExternalOutput")

    scale = float(D) ** -0.5

    with TileContext(nc) as tc:
        with (
            tc.tile_pool(name="const", bufs=1) as const,
            tc.tile_pool(name="ps_mm", bufs=5, space="PSUM") as ps_mm,
            tc.tile_pool(name="ps_sum", bufs=2, space="PSUM") as ps_sum,
            tc.tile_pool(name="dramtmp", bufs=1, space="DRAM") as dramtmp,
        ):
            # ---- constants -------------------------------------------------
            ones16 = const.tile([P, 1], F16)
            nc.vector.memset(ones16, 1.0)
            shift_b = const.tile([P, 1], F32)
            nc.vector.memset(shift_b, EXP_SHIFT)
            bqk = const.tile([P, 2 * DC], F32)
            nc.gpsimd.dma_start(
                out=bqk, in_=bqkv_d[0 : 2 * D].rearrange("(j p) -> p j", p=P)
            )
            bq_s = const.tile([P, DC], F32)
            nc.vector.tensor_scalar_mul(bq_s, bqk[:, 0:DC], scale)
            bv_b = const.tile([P, D], F32)
            nc.gpsimd.dma_start(
                out=bv_b, in_=bqkv_d[None, 2 * D : 3 * D].to_broadcast([P, D])
            )
            bout_b = const.tile([P, D], F32)
            nc.gpsimd.dma_start(out=bout_b, in_=bout_d[None, :].to_broadcast([P, D]))
            sums_sb = const.tile([1, SQ], F32)
            sumsT = const.tile([P, SQT], F32)
            rsum = const.tile([P, SQT], F32)

            with (
                tc.tile_pool(name="kq", bufs=1) as kq,
                tc.tile_pool(name="vpool", bufs=1) as vpool,
            ):
                qT = kq.tile([P, DC, SQ], F16)
                kT = kq.tile([P, DC, S], F16)
                v = vpool.tile([P, ST, D], F16)

                # ======== phase 1: load W / transpose x / QKV matmuls ======
                with tc.tile_pool(name="s1", bufs=1) as s1:
                    wq = s1.tile([P, DC, D], F16)
                    wk = s1.tile([P, DC, D], F16)
                    wv = s1.tile([P, DC, D], F16)
                    xTq = [s1.tile([P, DC, 512], F16, name=f"xTq{i}") for i in range(4)]

                    # Wv first: the v matmuls are gated on the least data
                    for dc in range(DC):
                        nc.sync.dma_start(
                            out=wv[:, dc, :],
                            in_=wqkv_d[dc * P : (dc + 1) * P, 2 * D : 3 * D],
                        )
                    for qt in range(2):  # xT quarters 0-1
                        for dc in range(DC):
                            nc.sync.dma_start_transpose(
                                xTq[qt][:, dc, :],
                                x_d[qt * 512 : (qt + 1) * 512, dc * P : (dc + 1) * P],
                            )
                    for dc in range(DC):  # Wq
                        nc.sync.dma_start(
                            out=wq[:, dc, :],
                            in_=wqkv_d[dc * P : (dc + 1) * P, 0:D],
                        )
                    for qt in range(2, 4):  # xT quarters 2-3
                        for dc in range(DC):
                            nc.sync.dma_start_transpose(
                                xTq[qt][:, dc, :],
                                x_d[qt * 512 : (qt + 1) * 512, dc * P : (dc + 1) * P],
                            )
                    for dc in range(DC):  # Wk
                        nc.sync.dma_start(
                            out=wk[:, dc, :],
                            in_=wqkv_d[dc * P : (dc + 1) * P, D : 2 * D],
                        )

                    # v natural [s, d]
                    for st in range(ST):
                        pss = [ps_mm.tile([P, 512], F32, tag="mm", name="mm") for _ in range(2)]
                        for dc in range(DC):
                            for oc in range(2):
                                nc.tensor.matmul(
                                    pss[oc],
                                    lhsT=xTq[st // 4][
                                        :, dc, (st % 4) * P : (st % 4 + 1) * P
                                    ],
                                    rhs=wv[:, dc, oc * 512 : (oc + 1) * 512],
                                    start=(dc == 0),
                                    stop=(dc == DC - 1),
                                )
                        for oc in range(2):
                            nc.vector.tensor_tensor(
                                out=v[:, st, oc * 512 : (oc + 1) * 512],
                                in0=pss[oc],
                                in1=bv_b[:, oc * 512 : (oc + 1) * 512],
                                op=mybir.AluOpType.add,
                            )

                    # qT [d, SQ] with 1/sqrt(D) folded in
                    for j in range(DC):
                        pss = [ps_mm.tile([P, 512], F32, tag="mm", name="mm") for _ in range(NQ)]
                        for dc in range(DC):
                            for sc in range(NQ):
                                nc.tensor.matmul(
                                    pss[sc],
                                    lhsT=wq[:, dc, j * P : (j + 1) * P],
                                    rhs=xTq[sc][:, dc, :],
                                    start=(dc == 0),
                                    stop=(dc == DC - 1),
                                )
                        for sc in range(NQ):
                            nc.scalar.activation(
                                qT[:, j, sc * 512 : (sc + 1) * 512],
                                pss[sc],
                                mybir.ActivationFunctionType.Identity,
                                bias=bq_s[:, j : j + 1],
                                scale=scale,
                            )
                    # kT [d, S]
                    for j in range(DC):
                        for half in range(2):  # 2 x 2 chunks to bound live psums
                            pss = [
                                ps_mm.tile([P, 512], F32, tag="mm", name="mm") for _ in range(2)
                            ]
                            for dc in range(DC):
                                for i, sc in enumerate((2 * half, 2 * half + 1)):
                                    nc.tensor.matmul(
                                        pss[i],
                                        lhsT=wk[:, dc, j * P : (j + 1) * P],
                                        rhs=xTq[sc][:, dc, :],
                                        start=(dc == 0),
                                        stop=(dc == DC - 1),
                                    )
                            for i, sc in enumerate((2 * half, 2 * half + 1)):
                                nc.scalar.activation(
                                    kT[:, j, sc * 512 : (sc + 1) * 512],
                                    pss[i],
                                    mybir.ActivationFunctionType.Identity,
                                    bias=bqk[:, DC + j : DC + j + 1],
                                    scale=1.0,
                                )
                # ======== phase 2: scores/exp, sums, PV, out proj ==========
                with (
                    tc.tile_pool(name="s2", bufs=1) as s2,
                    tc.tile_pool(name="outbuf", bufs=2) as outbuf,
                ):
                    wout = s2.tile([P, DC, D], F16)
                    for dc in range(DC):
                        nc.sync.dma_start(
                            out=wout[:, dc, :], in_=wout_d[dc * P : (dc + 1) * P, :]
                        )

                    # PT[st][s, sq] = exp(k qT^T + shift) f16, plus row sums
                    PT = [s2.tile([P, SQ], F16, name=f"PT{st}") for st in range(ST)]
                    ps_sums = [ps_sum.tile([1, 512], F32, name="psum_s") for _ in range(NQ)]
                    for st in range(ST):
                        pss = [ps_mm.tile([P, 512], F32, tag="mm", name="mm") for _ in range(NQ)]
                        for dc in range(DC):
                            for sc in range(NQ):
                                nc.tensor.matmul(
                                    pss[sc],
                                    lhsT=kT[:, dc, st * P : (st + 1) * P],
                                    rhs=qT[:, dc, sc * 512 : (sc + 1) * 512],
                                    start=(dc == 0),
                                    stop=(dc == DC - 1),
                                )
                        for sc in range(NQ):
                            nc.scalar.activation(
                                PT[st][:, sc * 512 : (sc + 1) * 512],
                                pss[sc],
                                mybir.ActivationFunctionType.Exp,
                                bias=shift_b[:, 0:1],
                                scale=1.0,
                            )
                        for sc in range(NQ):
                            nc.tensor.matmul(
                                ps_sums[sc],
                                lhsT=ones16,
                                rhs=PT[st][:, sc * 512 : (sc + 1) * 512],
                                start=(st == 0),
                                stop=(st == ST - 1),
                            )
                    for sc in range(NQ):
                        nc.vector.tensor_copy(
                            out=sums_sb[:, sc * 512 : (sc + 1) * 512], in_=ps_sums[sc]
                        )
                    # [1, SQ] -> [128, SQT] via DRAM so sums line up with
                    # out partitions (SBUF APs cannot cross partitions)
                    sums_dram = dramtmp.tile([SQ], F32)
                    nc.sync.dma_start(out=sums_dram[None, :], in_=sums_sb)
                    nc.sync.dma_start(
                        out=sumsT, in_=sums_dram.rearrange("(t p) -> p t", p=P)
                    )
                    nc.vector.reciprocal(rsum, sumsT)

                    # attnT [d, sq] = v^T @ PT (unnormalized)
                    attnT = s2.tile([P, DC, SQ], F16)
                    for dc in range(DC):
                        pss = [ps_mm.tile([P, 512], F32, tag="mm", name="mm") for _ in range(NQ)]
                        for st in range(ST):
                            for sc in range(NQ):
                                nc.tensor.matmul(
                                    pss[sc],
                                    lhsT=v[:, st, dc * P : (dc + 1) * P],
                                    rhs=PT[st][:, sc * 512 : (sc + 1) * 512],
                                    start=(st == 0),
                                    stop=(st == ST - 1),
                                )
                        for sc in range(NQ):
                            nc.vector.tensor_copy(
                                out=attnT[:, dc, sc * 512 : (sc + 1) * 512],
                                in_=pss[sc],
                            )

                    # out[sq, d] = (attnT^T @ Wout) * rsum + bout
                    for sqt in range(SQT):
                        o_sb = outbuf.tile([P, D], F32, tag="o_sb")
                        pss = [ps_mm.tile([P, 512], F32, tag="mm", name="mm") for _ in range(2)]
                        for dc in range(DC):
                            for oc in range(2):
                                nc.tensor.matmul(
                                    pss[oc],
                                    lhsT=attnT[:, dc, sqt * P : (sqt + 1) * P],
                                    rhs=wout[:, dc, oc * 512 : (oc + 1) * 512],
                                    start=(dc == 0),
                                    stop=(dc == DC - 1),
                                )
                        for oc in range(2):
                            nc.vector.scalar_tensor_tensor(
                                out=o_sb[:, oc * 512 : (oc + 1) * 512],
                                in0=pss[oc],
                                scalar=rsum[:, sqt : sqt + 1],
                                in1=bout_b[:, oc * 512 : (oc + 1) * 512],
                                op0=mybir.AluOpType.mult,
                                op1=mybir.AluOpType.add,
                            )
                        for oc in range(2):
                            nc.sync.dma_start(
                                out=out_d[
                                    sqt * P : (sqt + 1) * P, oc * 512 : (oc + 1) * 512
                                ],
                                in_=o_sb[:, oc * 512 : (oc + 1) * 512],
                            )

    nc.finalize()
    return nc


def kernel(x, Wqkv, bqkv, Wout, bout):
    from concourse.bass_utils import run_bass_kernel_spmd

    x = np.ascontiguousarray(x, dtype=np.float32)
    bqkv = np.ascontiguousarray(bqkv, dtype=np.float32)
    bout = np.ascontiguousarray(bout, dtype=np.float32)
    wqkv16 = np.ascontiguousarray(np.asarray(Wqkv, dtype=np.float32).astype(np.float16))
    wout16 = np.ascontiguousarray(np.asarray(Wout, dtype=np.float32).astype(np.float16))
    B = x.shape[0]

    nc = _build_core_program()

    in_maps = []
    for c in range(8):
        b, h = c // 2, c % 2
        # rotate the sequence so this core's queries are rows 0:SQ
        xs = np.concatenate([x[b, h * SQ :], x[b, : h * SQ]], axis=0)
        in_maps.append(
            {
                "x16": np.ascontiguousarray(xs.astype(np.float16)),
                "Wqkv16": wqkv16,
                "bqkv": bqkv,
                "Wout16": wout16,
                "bout": bout,
            }
        )

    res = run_bass_kernel_spmd(nc, in_maps, core_ids=list(range(8)))

    out = np.empty((B, S, D), dtype=np.float32)
    for c in range(8):
        b, h = c // 2, c % 2
        out[b, h * SQ : (h + 1) * SQ, :] = res.results[c]["out"]
    return out


# revision 13
# speedup vs baseline: 1.0216x; 1.0216x over previous
"""Trainium2 Bass kernel for single-head attention:
out = softmax((x@Wq)(x@Wk)^T / sqrt(D)) (x@Wv) @ Wout + bout, per batch.

Shapes: x [4, 2048, 1024], Wqkv [1024, 3072], Wout [1024, 1024].

Sharding: 8 cores = 4 batches x 2 query-halves. Each core gets the full
sequence of its batch (rotated so its 1024 queries are rows 0:1023 --
attention is permutation-invariant over key/value positions), computes
QKV for the whole sequence, attention + output projection for its 1024
queries, and writes a [1024, 1024] slice of the output. No collectives.

x and the weight matrices are pre-cast to fp16 host-side (same rounding
the on-chip cast would apply); all matmuls run fp16 (full PE rate) with
fp32 PSUM accumulation. x^T is produced by DMA xbar transpose. Softmax
exp uses a constant -4 logit shift so fp16 never overflows (the shift
cancels in the normalization).
"""

import sys

if "/opt/trn_rl_repo" not in sys.path:
    sys.path.insert(0, "/opt/trn_rl_repo")

import numpy as np

import concourse.bass as bass
import concourse.mybir as mybir
from concourse import bacc
from concourse.tile import TileContext

P = 128
D = 1024          # d_model
S = 2048          # full sequence per batch
SQ = 1024         # queries per core
DC = D // P       # 8 d-chunks
ST = S // P       # 16 sequence tiles
NQ = SQ // 512    # 2 query 512-chunks
NK = S // 512     # 4 sequence 512-chunks
SQT = SQ // P     # 8 query 128-tiles

F32 = mybir.dt.float32
F16 = mybir.dt.float16
EXP_SHIFT = -4.0  # softmax logit shift (cancels in normalization)


def _build_core_program():
    nc = bacc.Bacc()

    x_d = nc.dram_tensor("x16", [S, D], F16, kind="ExternalInput")
    wqkv_d = nc.dram_tensor("Wqkv16", [D, 3 * D], F16, kind="ExternalInput")
    bqkv_d = nc.dram_tensor("bqkv", [3 * D], F32, kind="ExternalInput")
    wout_d = nc.dram_tensor("Wout16", [D, D], F16, kind="ExternalInput")
    bout_d = nc.dram_tensor("bout", [D], F32, kind="ExternalInput")
    out_d = nc.dram_tensor("out", [SQ, D], F32, kind="ExternalOutput")

    scale = float(D) ** -0.5

    with TileContext(nc) as tc:
        with (
            tc.tile_pool(name="const", bufs=1) as const,
            tc.tile_pool(name="ps_mm", bufs=5, space="PSUM") as ps_mm,
            tc.tile_pool(name="ps_sum", bufs=2, space="PSUM") as ps_sum,
            tc.tile_pool(name="dramtmp", bufs=1, space="DRAM") as dramtmp,
        ):
            # ---- constants -------------------------------------------------
            ones16 = const.tile([P, 1], F16)
            nc.vector.memset(ones16, 1.0)
            shift_b = const.tile([P, 1], F32)
            nc.vector.memset(shift_b, EXP_SHIFT)
            bqk = const.tile([P, 2 * DC], F32)
            nc.gpsimd.dma_start(
                out=bqk, in_=bqkv_d[0 : 2 * D].rearrange("(j p) -> p j", p=P)
            )
            bq_s = const.tile([P, DC], F32)
            nc.vector.tensor_scalar_mul(bq_s, bqk[:, 0:DC], scale)
            bv_b = const.tile([P, D], F32)
            nc.gpsimd.dma_start(
                out=bv_b, in_=bqkv_d[None, 2 * D : 3 * D].to_broadcast([P, D])
            )
            bout_b = const.tile([P, D], F32)
            nc.gpsimd.dma_start(out=bout_b, in_=bout_d[None, :].to_broadcast([P, D]))
            sums_sb = const.tile([1, SQ], F32)
            sumsT = const.tile([P, SQT], F32)
            rsum = const.tile([P, SQT], F32)

            with (
                tc.tile_pool(name="kq", bufs=1) as kq,
                tc.tile_pool(name="vpool", bufs=1) as vpool,
            ):
                qT = kq.tile([P, DC, SQ], F16)
                kT = kq.tile([P, DC, S], F16)
                v = vpool.tile([P, ST, D], F16)

                # ======== phase 1: load W / transpose x / QKV matmuls ======
                with tc.tile_pool(name="s1", bufs=1) as s1:
                    wq = s1.tile([P, DC, D], F16)
                    wk = s1.tile([P, DC, D], F16)
                    wv = s1.tile([P, DC, D], F16)
                    xTh = [s1.tile([P, DC, 1024], F16, name=f"xTh{i}") for i in range(2)]

                    # Wv first (v matmuls are gated on the least data); one
                    # batched DMA per weight matrix; x transposes split
                    # across the two HWDGE issue sequencers (sync + scalar)
                    nc.sync.dma_start(
                        out=wv,
                        in_=wqkv_d[:, 2 * D : 3 * D].rearrange(
                            "(dc p) n -> p dc n", p=P
                        ),
                    )
                    for h in range(2):
                        for dc in range(DC):
                            nc.sync.dma_start_transpose(
                                xTh[h][:, dc, :],
                                x_d[h * 1024 : (h + 1) * 1024, dc * P : (dc + 1) * P],
                            )
                        if h == 0:
                            nc.sync.dma_start(
                                out=wq,
                                in_=wqkv_d[:, 0:D].rearrange("(dc p) n -> p dc n", p=P),
                            )
                    nc.sync.dma_start(
                        out=wk,
                        in_=wqkv_d[:, D : 2 * D].rearrange("(dc p) n -> p dc n", p=P),
                    )

                    # v natural [s, d]
                    for st in range(ST):
                        pss = [ps_mm.tile([P, 512], F32, tag="mm", name="mm") for _ in range(2)]
                        for dc in range(DC):
                            for oc in range(2):
                                nc.tensor.matmul(
                                    pss[oc],
                                    lhsT=xTh[st // 8][
                                        :, dc, (st % 8) * P : (st % 8 + 1) * P
                                    ],
                                    rhs=wv[:, dc, oc * 512 : (oc + 1) * 512],
                                    start=(dc == 0),
                                    stop=(dc == DC - 1),
                                )
                        for oc in range(2):
                            nc.vector.tensor_tensor(
                                out=v[:, st, oc * 512 : (oc + 1) * 512],
                                in0=pss[oc],
                                in1=bv_b[:, oc * 512 : (oc + 1) * 512],
                                op=mybir.AluOpType.add,
                            )

                    # qT [d, SQ] with 1/sqrt(D) folded in
                    for j in range(DC):
                        pss = [ps_mm.tile([P, 512], F32, tag="mm", name="mm") for _ in range(NQ)]
                        for dc in range(DC):
                            for sc in range(NQ):
                                nc.tensor.matmul(
                                    pss[sc],
                                    lhsT=wq[:, dc, j * P : (j + 1) * P],
                                    rhs=xTh[sc // 2][:, dc, (sc % 2) * 512 : (sc % 2 + 1) * 512],
                                    start=(dc == 0),
                                    stop=(dc == DC - 1),
                                )
                        for sc in range(NQ):
                            nc.scalar.activation(
                                qT[:, j, sc * 512 : (sc + 1) * 512],
                                pss[sc],
                                mybir.ActivationFunctionType.Identity,
                                bias=bq_s[:, j : j + 1],
                                scale=scale,
                            )
                    # kT [d, S]
                    for j in range(DC):
                        for half in range(2):  # 2 x 2 chunks to bound live psums
                            pss = [
                                ps_mm.tile([P, 512], F32, tag="mm", name="mm") for _ in range(2)
                            ]
                            for dc in range(DC):
                                for i, sc in enumerate((2 * half, 2 * half + 1)):
                                    nc.tensor.matmul(
                                        pss[i],
                                        lhsT=wk[:, dc, j * P : (j + 1) * P],
                                        rhs=xTh[sc // 2][:, dc, (sc % 2) * 512 : (sc % 2 + 1) * 512],
                                        start=(dc == 0),
                                        stop=(dc == DC - 1),
                                    )
                            for i, sc in enumerate((2 * half, 2 * half + 1)):
                                nc.scalar.activation(
                                    kT[:, j, sc * 512 : (sc + 1) * 512],
                                    pss[i],
                                    mybir.ActivationFunctionType.Identity,
                                    bias=bqk[:, DC + j : DC + j + 1],
                                    scale=1.0,
                                )
                # ======== phase 2: scores/exp, sums, PV, out proj ==========
                with (
                    tc.tile_pool(name="s2", bufs=1) as s2,
                    tc.tile_pool(name="outbuf", bufs=2) as outbuf,
                ):
                    wout = s2.tile([P, DC, D], F16)
                    nc.sync.dma_start(
                        out=wout, in_=wout_d.rearrange("(dc p) n -> p dc n", p=P)
                    )

                    # PT[st][s, sq] = exp(k qT^T + shift) f16, plus row sums
                    PT = [s2.tile([P, SQ], F16, name=f"PT{st}") for st in range(ST)]
                    ps_sums = [ps_sum.tile([1, 512], F32, name="psum_s") for _ in range(NQ)]
                    for st in range(ST):
                        pss = [ps_mm.tile([P, 512], F32, tag="mm", name="mm") for _ in range(NQ)]
                        for dc in range(DC):
                            for sc in range(NQ):
                                nc.tensor.matmul(
                                    pss[sc],
                                    lhsT=kT[:, dc, st * P : (st + 1) * P],
                                    rhs=qT[:, dc, sc * 512 : (sc + 1) * 512],
                                    start=(dc == 0),
                                    stop=(dc == DC - 1),
                                )
                        for sc in range(NQ):
                            nc.scalar.activation(
                                PT[st][:, sc * 512 : (sc + 1) * 512],
                                pss[sc],
                                mybir.ActivationFunctionType.Exp,
                                bias=shift_b[:, 0:1],
                                scale=1.0,
                            )
                        for sc in range(NQ):
                            nc.tensor.matmul(
                                ps_sums[sc],
                                lhsT=ones16,
                                rhs=PT[st][:, sc * 512 : (sc + 1) * 512],
                                start=(st == 0),
                                stop=(st == ST - 1),
                            )
                    for sc in range(NQ):
                        nc.vector.tensor_copy(
                            out=sums_sb[:, sc * 512 : (sc + 1) * 512], in_=ps_sums[sc]
                        )
                    # [1, SQ] -> [128, SQT] via DRAM so sums line up with
                    # out partitions (SBUF APs cannot cross partitions)
                    sums_dram = dramtmp.tile([SQ], F32)
                    nc.sync.dma_start(out=sums_dram[None, :], in_=sums_sb)
                    nc.sync.dma_start(
                        out=sumsT, in_=sums_dram.rearrange("(t p) -> p t", p=P)
                    )
                    nc.vector.reciprocal(rsum, sumsT)

                    # attnT [d, sq] = v^T @ PT (unnormalized)
                    attnT = s2.tile([P, DC, SQ], F16)
                    for dc in range(DC):
                        pss = [ps_mm.tile([P, 512], F32, tag="mm", name="mm") for _ in range(NQ)]
                        for st in range(ST):
                            for sc in range(NQ):
                                nc.tensor.matmul(
                                    pss[sc],
                                    lhsT=v[:, st, dc * P : (dc + 1) * P],
                                    rhs=PT[st][:, sc * 512 : (sc + 1) * 512],
                                    start=(st == 0),
                                    stop=(st == ST - 1),
                                )
                        for sc in range(NQ):
                            nc.vector.tensor_copy(
                                out=attnT[:, dc, sc * 512 : (sc + 1) * 512],
                                in_=pss[sc],
                            )

                    # out[sq, d] = (attnT^T @ Wout) * rsum + bout
                    for sqt in range(SQT):
                        o_sb = outbuf.tile([P, D], F32, tag="o_sb")
                        pss = [ps_mm.tile([P, 512], F32, tag="mm", name="mm") for _ in range(2)]
                        for dc in range(DC):
                            for oc in range(2):
                                nc.tensor.matmul(
                                    pss[oc],
                                    lhsT=attnT[:, dc, sqt * P : (sqt + 1) * P],
                                    rhs=wout[:, dc, oc * 512 : (oc + 1) * 512],
                                    start=(dc == 0),
                                    stop=(dc == DC - 1),
                                )
                        for oc in range(2):
                            nc.vector.scalar_tensor_tensor(
                                out=o_sb[:, oc * 512 : (oc + 1) * 512],
                                in0=pss[oc],
                                scalar=rsum[:, sqt : sqt + 1],
                                in1=bout_b[:, oc * 512 : (oc + 1) * 512],
                                op0=mybir.AluOpType.mult,
                                op1=mybir.AluOpType.add,
                            )
                        for oc in range(2):
                            nc.sync.dma_start(
                                out=out_d[
                                    sqt * P : (sqt + 1) * P, oc * 512 : (oc + 1) * 512
                                ],
                                in_=o_sb[:, oc * 512 : (oc + 1) * 512],
                            )

    nc.finalize()
    return nc


def kernel(x, Wqkv, bqkv, Wout, bout):
    from concourse.bass_utils import run_bass_kernel_spmd

    x = np.ascontiguousarray(x, dtype=np.float32)
    bqkv = np.ascontiguousarray(bqkv, dtype=np.float32)
    bout = np.ascontiguousarray(bout, dtype=np.float32)
    wqkv16 = np.ascontiguousarray(np.asarray(Wqkv, dtype=np.float32).astype(np.float16))
    wout16 = np.ascontiguousarray(np.asarray(Wout, dtype=np.float32).astype(np.float16))
    B = x.shape[0]

    nc = _build_core_program()

    in_maps = []
    for c in range(8):
        b, h = c // 2, c % 2
        # rotate the sequence so this core's queries are rows 0:SQ
        xs = np.concatenate([x[b, h * SQ :], x[b, : h * SQ]], axis=0)
        in_maps.append(
            {
                "x16": np.ascontiguousarray(xs.astype(np.float16)),
                "Wqkv16": wqkv16,
                "bqkv": bqkv,
                "Wout16": wout16,
                "bout": bout,
            }
        )

    res = run_bass_kernel_spmd(nc, in_maps, core_ids=list(range(8)))

    out = np.empty((B, S, D), dtype=np.float32)
    for c in range(8):
        b, h = c // 2, c % 2
        out[b, h * SQ : (h + 1) * SQ, :] = res.results[c]["out"]
    return out


# revision 15
# speedup vs baseline: 1.0419x; 1.0199x over previous
"""Trainium2 Bass kernel for single-head attention:
out = softmax((x@Wq)(x@Wk)^T / sqrt(D)) (x@Wv) @ Wout + bout, per batch.

Shapes: x [4, 2048, 1024], Wqkv [1024, 3072], Wout [1024, 1024].

Sharding: 8 cores = 4 batches x 2 query-halves. Each core gets the full
sequence of its batch (rotated so its 1024 queries are rows 0:1023 --
attention is permutation-invariant over key/value positions), computes
QKV for the whole sequence, attention + output projection for its 1024
queries, and writes a [1024, 1024] slice of the output. No collectives.

x and the weight matrices are pre-cast to fp16 host-side (same rounding
the on-chip cast would apply); all matmuls run fp16 (full PE rate) with
fp32 PSUM accumulation. x^T is produced by DMA xbar transpose. Softmax
exp uses a constant -4 logit shift so fp16 never overflows (the shift
cancels in the normalization).
"""

import sys

if "/opt/trn_rl_repo" not in sys.path:
    sys.path.insert(0, "/opt/trn_rl_repo")

import numpy as np

import concourse.bass as bass
import concourse.mybir as mybir
from concourse import bacc
from concourse.tile import TileContext

P = 128
D = 1024          # d_model
S = 2048          # full sequence per batch
SQ = 1024         # queries per core
DC = D // P       # 8 d-chunks
ST = S // P       # 16 sequence tiles
NQ = SQ // 512    # 2 query 512-chunks
NK = S // 512     # 4 sequence 512-chunks
SQT = SQ // P     # 8 query 128-tiles

F32 = mybir.dt.float32
F16 = mybir.dt.float16
EXP_SHIFT = -4.0  # softmax logit shift (cancels in normalization)


def _build_core_program():
    nc = bacc.Bacc()

    x_d = nc.dram_tensor("x16", [S, D], F16, kind="ExternalInput")
    wqkv_d = nc.dram_tensor("Wqkv16", [D, 3 * D], F16, kind="ExternalInput")
    bqkv_d = nc.dram_tensor("bqkv", [3 * D], F32, kind="ExternalInput")
    wout_d = nc.dram_tensor("Wout16", [D, D], F16, kind="ExternalInput")
    bout_d = nc.dram_tensor("bout", [D], F32, kind="ExternalInput")
    out_d = nc.dram_tensor("out", [SQ, D], F32, kind="ExternalOutput")

    scale = float(D) ** -0.5

    with TileContext(nc) as tc:
        with (
            tc.tile_pool(name="const", bufs=1) as const,
            tc.tile_pool(name="ps_mm", bufs=5, space="PSUM") as ps_mm,
            tc.tile_pool(name="ps_sum", bufs=2, space="PSUM") as ps_sum,
            tc.tile_pool(name="dramtmp", bufs=1, space="DRAM") as dramtmp,
        ):
            # ---- constants -------------------------------------------------
            ones16 = const.tile([P, 1], F16)
            nc.vector.memset(ones16, 1.0)
            shift_b = const.tile([P, 1], F32)
            nc.vector.memset(shift_b, EXP_SHIFT)
            bqk = const.tile([P, 2 * DC], F32)
            nc.gpsimd.dma_start(
                out=bqk, in_=bqkv_d[0 : 2 * D].rearrange("(j p) -> p j", p=P)
            )
            bq_s = const.tile([P, DC], F32)
            nc.vector.tensor_scalar_mul(bq_s, bqk[:, 0:DC], scale)
            bv_b = const.tile([P, D], F32)
            nc.gpsimd.dma_start(
                out=bv_b, in_=bqkv_d[None, 2 * D : 3 * D].to_broadcast([P, D])
            )
            bout_b = const.tile([P, D], F32)
            nc.gpsimd.dma_start(out=bout_b, in_=bout_d[None, :].to_broadcast([P, D]))
            sums_sb = const.tile([1, SQ], F32)
            sumsT = const.tile([P, SQT], F32)
            rsum = const.tile([P, SQT], F32)

            # HAM warmup: keep the PE busy while the first inputs stream in,
            # so real matmuls start at 2.4 GHz instead of 1.2 GHz
            warm = const.tile([P, 512], F16)
            nc.vector.memset(warm, 0.0)
            ps_warm = ps_mm.tile([P, 512], F32, tag="mm", name="ps_warm")
            for _ in range(18):
                nc.tensor.matmul(
                    ps_warm, lhsT=warm[:, 0:P], rhs=warm, start=True, stop=True
                )

            with (
                tc.tile_pool(name="kq", bufs=1) as kq,
                tc.tile_pool(name="vpool", bufs=1) as vpool,
            ):
                qT = kq.tile([P, DC, SQ], F16)
                kT = kq.tile([P, DC, S], F16)
                v = vpool.tile([P, ST, D], F16)

                # ======== phase 1: load W / transpose x / QKV matmuls ======
                with tc.tile_pool(name="s1", bufs=1) as s1:
                    wq = s1.tile([P, DC, D], F16)
                    wk = s1.tile([P, DC, D], F16)
                    wv = s1.tile([P, DC, D], F16)
                    xTh = [s1.tile([P, DC, 1024], F16, name=f"xTh{i}") for i in range(2)]

                    # Wv first (v matmuls are gated on the least data); one
                    # batched DMA per weight matrix; x transposes split
                    # across the two HWDGE issue sequencers (sync + scalar)
                    for g in range(4):
                        nc.sync.dma_start(
                            out=wv[:, 2 * g : 2 * g + 2, :],
                            in_=wqkv_d[
                                2 * g * P : (2 * g + 2) * P, 2 * D : 3 * D
                            ].rearrange("(dc p) n -> p dc n", p=P),
                        )
                    for h in range(2):
                        for dc in range(DC):
                            nc.sync.dma_start_transpose(
                                xTh[h][:, dc, :],
                                x_d[h * 1024 : (h + 1) * 1024, dc * P : (dc + 1) * P],
                            )
                        if h == 0:
                            nc.sync.dma_start(
                                out=wq,
                                in_=wqkv_d[:, 0:D].rearrange("(dc p) n -> p dc n", p=P),
                            )
                    nc.sync.dma_start(
                        out=wk,
                        in_=wqkv_d[:, D : 2 * D].rearrange("(dc p) n -> p dc n", p=P),
                    )

                    # v natural [s, d]
                    for st in range(ST):
                        pss = [ps_mm.tile([P, 512], F32, tag="mm", name="mm") for _ in range(2)]
                        for dc in range(DC):
                            for oc in range(2):
                                nc.tensor.matmul(
                                    pss[oc],
                                    lhsT=xTh[st // 8][
                                        :, dc, (st % 8) * P : (st % 8 + 1) * P
                                    ],
                                    rhs=wv[:, dc, oc * 512 : (oc + 1) * 512],
                                    start=(dc == 0),
                                    stop=(dc == DC - 1),
                                )
                        for oc in range(2):
                            nc.vector.tensor_tensor(
                                out=v[:, st, oc * 512 : (oc + 1) * 512],
                                in0=pss[oc],
                                in1=bv_b[:, oc * 512 : (oc + 1) * 512],
                                op=mybir.AluOpType.add,
                            )

                    # qT [d, SQ] with 1/sqrt(D) folded in
                    for j in range(DC):
                        pss = [ps_mm.tile([P, 512], F32, tag="mm", name="mm") for _ in range(NQ)]
                        for dc in range(DC):
                            for sc in range(NQ):
                                nc.tensor.matmul(
                                    pss[sc],
                                    lhsT=wq[:, dc, j * P : (j + 1) * P],
                                    rhs=xTh[sc // 2][:, dc, (sc % 2) * 512 : (sc % 2 + 1) * 512],
                                    start=(dc == 0),
                                    stop=(dc == DC - 1),
                                )
                        for sc in range(NQ):
                            nc.scalar.activation(
                                qT[:, j, sc * 512 : (sc + 1) * 512],
                                pss[sc],
                                mybir.ActivationFunctionType.Identity,
                                bias=bq_s[:, j : j + 1],
                                scale=scale,
                            )
                    # kT [d, S]
                    for j in range(DC):
                        for half in range(2):  # 2 x 2 chunks to bound live psums
                            pss = [
                                ps_mm.tile([P, 512], F32, tag="mm", name="mm") for _ in range(2)
                            ]
                            for dc in range(DC):
                                for i, sc in enumerate((2 * half, 2 * half + 1)):
                                    nc.tensor.matmul(
                                        pss[i],
                                        lhsT=wk[:, dc, j * P : (j + 1) * P],
                                        rhs=xTh[sc // 2][:, dc, (sc % 2) * 512 : (sc % 2 + 1) * 512],
                                        start=(dc == 0),
                                        stop=(dc == DC - 1),
                                    )
                            for i, sc in enumerate((2 * half, 2 * half + 1)):
                                nc.scalar.activation(
                                    kT[:, j, sc * 512 : (sc + 1) * 512],
                                    pss[i],
                                    mybir.ActivationFunctionType.Identity,
                                    bias=bqk[:, DC + j : DC + j + 1],
                                    scale=1.0,
                                )
                # ======== phase 2: scores/exp, sums, PV, out proj ==========
                with (
                    tc.tile_pool(name="s2", bufs=1) as s2,
                    tc.tile_pool(name="outbuf", bufs=2) as outbuf,
                ):
                    wout = s2.tile([P, DC, D], F16)
                    nc.sync.dma_start(
                        out=wout, in_=wout_d.rearrange("(dc p) n -> p dc n", p=P)
                    )

                    # PT[st][s, sq] = exp(k qT^T + shift) f16, plus row sums
                    PT = [s2.tile([P, SQ], F16, name=f"PT{st}") for st in range(ST)]
                    ps_sums = [ps_sum.tile([1, 512], F32, name="psum_s") for _ in range(NQ)]
                    for st in range(ST):
                        pss = [ps_mm.tile([P, 512], F32, tag="mm", name="mm") for _ in range(NQ)]
                        for dc in range(DC):
                            for sc in range(NQ):
                                nc.tensor.matmul(
                                    pss[sc],
                                    lhsT=kT[:, dc, st * P : (st + 1) * P],
                                    rhs=qT[:, dc, sc * 512 : (sc + 1) * 512],
                                    start=(dc == 0),
                                    stop=(dc == DC - 1),
                                )
                        for sc in range(NQ):
                            nc.scalar.activation(
                                PT[st][:, sc * 512 : (sc + 1) * 512],
                                pss[sc],
                                mybir.ActivationFunctionType.Exp,
                                bias=shift_b[:, 0:1],
                                scale=1.0,
                            )
                        for sc in range(NQ):
                            nc.tensor.matmul(
                                ps_sums[sc],
                                lhsT=ones16,
                                rhs=PT[st][:, sc * 512 : (sc + 1) * 512],
                                start=(st == 0),
                                stop=(st == ST - 1),
                            )
                    for sc in range(NQ):
                        nc.vector.tensor_copy(
                            out=sums_sb[:, sc * 512 : (sc + 1) * 512], in_=ps_sums[sc]
                        )
                    # [1, SQ] -> [128, SQT] via DRAM so sums line up with
                    # out partitions (SBUF APs cannot cross partitions)
                    sums_dram = dramtmp.tile([SQ], F32)
                    nc.sync.dma_start(out=sums_dram[None, :], in_=sums_sb)
                    nc.sync.dma_start(
                        out=sumsT, in_=sums_dram.rearrange("(t p) -> p t", p=P)
                    )
                    nc.vector.reciprocal(rsum, sumsT)

                    # attnT [d, sq] = v^T @ PT (unnormalized)
                    attnT = s2.tile([P, DC, SQ], F16)
                    for dc in range(DC):
                        pss = [ps_mm.tile([P, 512], F32, tag="mm", name="mm") for _ in range(NQ)]
                        for st in range(ST):
                            for sc in range(NQ):
                                nc.tensor.matmul(
                                    pss[sc],
                                    lhsT=v[:, st, dc * P : (dc + 1) * P],
                                    rhs=PT[st][:, sc * 512 : (sc + 1) * 512],
                                    start=(st == 0),
                                    stop=(st == ST - 1),
                                )
                        for sc in range(NQ):
                            nc.vector.tensor_copy(
                                out=attnT[:, dc, sc * 512 : (sc + 1) * 512],
                                in_=pss[sc],
                            )

                    # out[sq, d] = (attnT^T @ Wout) * rsum + bout
                    for sqt in range(SQT):
                        o_sb = outbuf.tile([P, D], F32, tag="o_sb")
                        pss = [ps_mm.tile([P, 512], F32, tag="mm", name="mm") for _ in range(2)]
                        for dc in range(DC):
                            for oc in range(2):
                                nc.tensor.matmul(
                                    pss[oc],
                                    lhsT=attnT[:, dc, sqt * P : (sqt + 1) * P],
                                    rhs=wout[:, dc, oc * 512 : (oc + 1) * 512],
                                    start=(dc == 0),
                                    stop=(dc == DC - 1),
                                )
                        for oc in range(2):
                            nc.vector.scalar_tensor_tensor(
                                out=o_sb[:, oc * 512 : (oc + 1) * 512],
                                in0=pss[oc],
                                scalar=rsum[:, sqt : sqt + 1],
                                in1=bout_b[:, oc * 512 : (oc + 1) * 512],
                                op0=mybir.AluOpType.mult,
                                op1=mybir.AluOpType.add,
                            )
                        for oc in range(2):
                            nc.sync.dma_start(
                                out=out_d[
                                    sqt * P : (sqt + 1) * P, oc * 512 : (oc + 1) * 512
                                ],
                                in_=o_sb[:, oc * 512 : (oc + 1) * 512],
                            )

    nc.finalize()
    return nc


def kernel(x, Wqkv, bqkv, Wout, bout):
    from concourse.bass_utils import run_bass_kernel_spmd

    x = np.ascontiguousarray(x, dtype=np.float32)
    bqkv = np.ascontiguousarray(bqkv, dtype=np.float32)
    bout = np.ascontiguousarray(bout, dtype=np.float32)
    wqkv16 = np.ascontiguousarray(np.asarray(Wqkv, dtype=np.float32).astype(np.float16))
    wout16 = np.ascontiguousarray(np.asarray(Wout, dtype=np.float32).astype(np.float16))
    B = x.shape[0]

    nc = _build_core_program()

    in_maps = []
    for c in range(8):
        b, h = c // 2, c % 2
        # rotate the sequence so this core's queries are rows 0:SQ
        xs = np.concatenate([x[b, h * SQ :], x[b, : h * SQ]], axis=0)
        in_maps.append(
            {
                "x16": np.ascontiguousarray(xs.astype(np.float16)),
                "Wqkv16": wqkv16,
                "bqkv": bqkv,
                "Wout16": wout16,
                "bout": bout,
            }
        )

    res = run_bass_kernel_spmd(nc, in_maps, core_ids=list(range(8)))

    out = np.empty((B, S, D), dtype=np.float32)
    for c in range(8):
        b, h = c // 2, c % 2
        out[b, h * SQ : (h + 1) * SQ, :] = res.results[c]["out"]
    return out
